# revision 1
# baseline (speedup 1.0000x reference)
"""Trainium2 Bass kernel for nn_AutoEncoderLoss (two-level segment-mean MSE).

Strategy
--------
batch_index is sorted, so the N points split into `num_batches` contiguous
runs. The host finds the 32 run boundaries (np.searchsorted - O(B log N)) and
shards *whole batches* across the 8 cores (4 batches/core, sizes are
near-identical). Each batch range is laid out as a [128, T_pad] tile
(contiguous per partition), padded with clabel=255 (out-of-range -> one-hot
all zero) and reco=target=0.

On each core, for every batch range we compute a 128-bin weighted histogram
(cluster sums of (reco-target)^2, and counts) with a factored one-hot:
  h = clabel >> 3 (16 values), l = clabel & 7 (8 values)
  DVE builds bin-major "slabs" with constant-scalar compares (fast 4x mode):
    16x (h==H) bf16, 8x (l==L) bf16, 8x (l==L)*v bf16
  PE multiplies hi-slabs against lo-slabs 8 point-columns at a time:
    lhsT[128, 8*16] (8 chunks' hi one-hots), rhs[128, 8*16] (lo cnt|val),
    accumulating in PSUM. The 8 diagonal [16,16] blocks hold
    [counts | sums] per (H, L); off-diagonal blocks are ignored junk.
PSUM banks (one per batch range) are dumped to DRAM; the host folds the
8 diagonal blocks, assembles the [32, 128] segment sums/counts and does the
final O(B*C) masked-mean reduction.
"""

import math
import numpy as np
from contextlib import ExitStack

NCORES = 8
HI = 16  # hi one-hot bins (clabel >> 3)
LOB = 8  # lo one-hot bins (clabel & 7)
GROUP = 8  # point-columns per matmul (GROUP*HI = 128 = max stationary cols)
import os as _os
T_TILE = int(_os.environ.get("K_T_TILE", "640"))  # SBUF tile width
LOVAL_MODE = _os.environ.get("K_LOVAL", "mul")  # "mul" | "stt"
PAD_LABEL = 255  # out-of-range label: h=31 matches no hi bin
RB = 12582912.0  # 1.5 * 2**23, fp32 round-to-int bias

_prog_cache = {}
_last_run = {}  # stashed (nc, in_maps) from the latest kernel() call


def profile_hw(np_inputs=None, k1=4, k2=1004, pairs=10, verbose=False):
    """Measure steady-state HW ns per kernel iteration.

    Runs two hardware-loop variants (k1/k2 repeats of the full compute,
    Internal-DRAM inputs so no transfers) in interleaved pairs; the median
    of per-pair wall-clock differences divided by (k2-k1) cancels dispatch
    overhead and is robust to the time-shared device's slow patches.
    """
    import time
    from concourse.bass_utils import run_bass_kernel_spmd
    if not _last_run and np_inputs is not None:
        kernel(**np_inputs)
    T_pad, R = _last_run["key"]

    ncs = {}
    for k in (k1, k2):
        ck = ("prof", T_pad, R, k, "full")
        if ck not in _prog_cache:
            _prog_cache[ck] = _build_program(T_pad, R, repeat=k,
                                             internal_inputs=True)
        ncs[k] = _prog_cache[ck]

    def one(k):
        t0 = time.time()
        run_bass_kernel_spmd(ncs[k], [{} for _ in range(NCORES)],
                             list(range(NCORES)))
        return time.time() - t0

    one(k1)  # warm both NEFFs
    one(k2)
    diffs = []
    for _ in range(pairs):
        try:
            ta = one(k1)
            tb = one(k2)
        except Exception:  # transient device flake: skip pair
            time.sleep(2)
            continue
        diffs.append((tb - ta) / (k2 - k1) * 1e9)
    diffs.sort()
    if verbose:
        print("pair diffs (ns/iter):", [f"{d:.0f}" for d in diffs])
    return diffs[len(diffs) // 2] if diffs else float("nan")


def profile_stages(np_inputs=None, k1=4, k2=104, samples=4):
    """Per-stage steady-state times (us): dma, +dve, +act-repack, full."""
    if not _last_run and np_inputs is not None:
        kernel(**np_inputs)
    out = {}
    for stage in ("dma", "dve", "act", "full"):
        import importlib
        t1 = _timed_prof(k1, stage, samples)
        t2 = _timed_prof(k2, stage, samples)
        out[stage] = (t2 - t1) / (k2 - k1) * 1e6
    return out


def _timed_prof(k, stage, samples):
    import time
    from concourse.bass_utils import run_bass_kernel_spmd
    T_pad, R = _last_run["key"]
    ck = ("prof", T_pad, R, k, stage)
    if ck not in _prog_cache:
        _prog_cache[ck] = _build_program(T_pad, R, repeat=k,
                                         internal_inputs=True, stage=stage)
    nc = _prog_cache[ck]
    best = float("inf")
    for _ in range(samples):
        t0 = time.time()
        run_bass_kernel_spmd(nc, [{} for _ in range(NCORES)],
                             list(range(NCORES)))
        best = min(best, time.time() - t0)
    return best


def _build_program(T_pad, R, repeat=None, internal_inputs=False, stage="full"):
    """Build + compile the SPMD bass program for R ranges of T_pad columns.

    repeat: wrap the whole compute in a hardware For_i loop (profiling).
    internal_inputs: inputs become Internal DRAM scratch (garbage data, no
    host transfer) - timing is data-independent, used only for profiling.
    """
    import concourse.tile as tile
    from concourse import bacc, mybir

    f32 = mybir.dt.float32
    bf16 = mybir.dt.bfloat16
    i32 = mybir.dt.int32
    AT = mybir.ActivationFunctionType
    OP = mybir.AluOpType

    nc = bacc.Bacc("TRN2", target_bir_lowering=False, debug=False,
                   num_devices=NCORES)
    in_kind = "Internal" if internal_inputs else "ExternalInput"
    rec = nc.dram_tensor("rec", [128, R * T_pad], f32, kind=in_kind).ap()
    tar = nc.dram_tensor("tar", [128, R * T_pad], f32, kind=in_kind).ap()
    lab = nc.dram_tensor("lab", [128, R * T_pad], i32, kind=in_kind).ap()
    out = nc.dram_tensor("out", [128, R * 128], f32, kind="ExternalOutput").ap()

    tiles = []
    t0 = 0
    while t0 < T_pad:
        tw = min(T_TILE, T_pad - t0)
        tiles.append((t0, tw))
        t0 += tw
    n_mm = T_pad // GROUP  # one matmul per GROUP point-columns per range

    with tile.TileContext(nc) as tc, ExitStack() as ctx:
        io_pool = ctx.enter_context(tc.tile_pool(name="io", bufs=2))
        tmp_pool = ctx.enter_context(tc.tile_pool(name="tmp", bufs=2))
        slab_pool = ctx.enter_context(tc.tile_pool(name="slab", bufs=2))
        psum_pool = ctx.enter_context(tc.tile_pool(name="psum", bufs=1, space="PSUM"))
        out_pool = ctx.enter_context(tc.tile_pool(name="outp", bufs=2))

        psums = [psum_pool.tile([128, 128], f32, tag=f"ps{r}", name=f"ps{r}")
                 for r in range(R)] if stage == "full" else [None] * R

        if repeat is not None:
            ctx.enter_context(tc.For_i(0, repeat, 1))

        for r in range(R):
            base = r * T_pad
            mm_i = 0
            for (t0, tw) in tiles:
                rec_t = io_pool.tile([128, tw], f32, tag="rec")
                nc.sync.dma_start(out=rec_t[:], in_=rec[:, base + t0:base + t0 + tw])
                tar_t = io_pool.tile([128, tw], f32, tag="tar")
                nc.sync.dma_start(out=tar_t[:], in_=tar[:, base + t0:base + t0 + tw])
                lab_t = io_pool.tile([128, tw], i32, tag="lab")
                nc.sync.dma_start(out=lab_t[:], in_=lab[:, base + t0:base + t0 + tw])
                if stage == "dma":
                    continue

                # h = floor(clabel/8) via fp32 round-to-nearest bias trick:
                # RN(c*0.125 - 0.4375) == floor(c/8) exactly for c in [0,256).
                # (c - 3.5)*0.125 first; then +RB forces integer rounding, -RB
                # recovers h. RB +/- offsets must stay separate ops: ulp(RB)=1.
                hf = tmp_pool.tile([128, tw], f32, tag="hf")
                nc.vector.tensor_scalar(hf[:], lab_t[:], -3.5, 0.125,
                                        OP.add, OP.mult)
                hb = tmp_pool.tile([128, tw], bf16, tag="hb")
                nc.vector.tensor_scalar(hb[:], hf[:], RB, -RB, OP.add, OP.add)
                # l = clabel - 8*h
                lb = tmp_pool.tile([128, tw], bf16, tag="lb")
                nc.vector.scalar_tensor_tensor(lb[:], hb[:], -8.0, lab_t[:],
                                               OP.mult, OP.add)
                # v = (rec - tar)^2 (bf16), square on ScalarE
                d_t = tmp_pool.tile([128, tw], f32, tag="d")
                nc.vector.tensor_sub(d_t[:], rec_t[:], tar_t[:])
                vb = tmp_pool.tile([128, tw], bf16, tag="vb")
                nc.scalar.activation(vb[:], d_t[:], AT.Square)

                # bin-major slabs
                hic = slab_pool.tile([128, HI, tw], bf16, tag="hic")
                for h in range(HI):
                    nc.vector.tensor_scalar(hic[:, h, :], hb[:], float(h), None,
                                            OP.is_equal)
                lot = slab_pool.tile([128, 2 * LOB, tw], bf16, tag="lot")
                for l in range(LOB):
                    nc.vector.tensor_scalar(lot[:, l, :], lb[:], float(l), None,
                                            OP.is_equal)
                # (l==L)*v via tensor_tensor on the count slab: TT bf16 runs
                # 2x while fused scalar_tensor_tensor measures only 1x
                for l in range(LOB):
                    if LOVAL_MODE == "mul":
                        nc.vector.tensor_mul(lot[:, LOB + l, :], lot[:, l, :],
                                             vb[:])
                    else:
                        nc.vector.scalar_tensor_tensor(lot[:, LOB + l, :],
                                                       lb[:], float(l), vb[:],
                                                       OP.is_equal, OP.mult)

                if stage == "dve":
                    continue
                # repack hi-slabs to t-major on ScalarE: hic2[p, t*16+H]
                # -> contiguous 128-col LDWEIGHTS slices (FWL-eligible)
                hic2 = slab_pool.tile([128, tw, HI], bf16, tag="hic2")
                nc.scalar.copy(hic2[:], hic[:].transpose([0, 2, 1]))
                if stage == "act":
                    continue

                for g in range(tw // GROUP):
                    lhsT = hic2[:, g * GROUP:(g + 1) * GROUP, :]  # [128,8,16] contig
                    rhs = lot[:, :, g * GROUP:(g + 1) * GROUP].transpose([0, 2, 1])
                    nc.tensor.matmul(psums[r][:], lhsT, rhs,
                                     start=(mm_i == 0), stop=(mm_i == n_mm - 1))
                    mm_i += 1

            if stage == "full":
                ob = out_pool.tile([128, 128], f32, tag="ob")
                nc.vector.tensor_copy(ob[:], psums[r][:])
                nc.sync.dma_start(out=out[:, r * 128:(r + 1) * 128], in_=ob[:])

    nc.compile()
    return nc


def kernel(reco, target, clabel, batch_index, num_batches, num_clusters):
    from concourse.bass_utils import run_bass_kernel_spmd

    B = int(num_batches)
    C = int(num_clusters)
    assert C == HI * LOB, f"kernel hardcoded for 128 clusters, got {C}"
    assert B % NCORES == 0, f"num_batches {B} not divisible by {NCORES}"
    R = B // NCORES

    reco = np.ascontiguousarray(np.asarray(reco, dtype=np.float32).reshape(-1))
    target = np.ascontiguousarray(np.asarray(target, dtype=np.float32).reshape(-1))
    clabel = np.asarray(clabel).astype(np.int32).reshape(-1)
    batch_index = np.asarray(batch_index).reshape(-1)
    N = reco.shape[0]

    # host: batch run boundaries (batch_index is sorted)
    bnd = np.searchsorted(batch_index, np.arange(B + 1), side="left")
    lens = np.diff(bnd)
    t_len = (lens + 127) // 128  # columns per batch
    T_pad = int(-(-int(t_len.max()) // GROUP) * GROUP)
    T_pad = max(T_pad, GROUP)

    key = (T_pad, R)
    if key not in _prog_cache:
        _prog_cache[key] = _build_program(T_pad, R)
    nc = _prog_cache[key]

    # build per-core input buffers
    in_maps = []
    for m in range(NCORES):
        rec_buf = np.zeros((128, R * T_pad), dtype=np.float32)
        tar_buf = np.zeros((128, R * T_pad), dtype=np.float32)
        lab_buf = np.full((128, R * T_pad), PAD_LABEL, dtype=np.int32)
        for r in range(R):
            b = m * R + r
            s, e = int(bnd[b]), int(bnd[b + 1])
            n = e - s
            if n == 0:
                continue
            tpb = (n + 127) // 128  # columns used by this batch
            block = np.zeros(128 * tpb, dtype=np.float32)
            block[:n] = reco[s:e]
            rec_buf[:, r * T_pad:r * T_pad + tpb] = block.reshape(128, tpb)
            block = np.zeros(128 * tpb, dtype=np.float32)
            block[:n] = target[s:e]
            tar_buf[:, r * T_pad:r * T_pad + tpb] = block.reshape(128, tpb)
            lblock = np.full(128 * tpb, PAD_LABEL, dtype=np.int32)
            lblock[:n] = clabel[s:e]
            lab_buf[:, r * T_pad:r * T_pad + tpb] = lblock.reshape(128, tpb)
        in_maps.append({"rec": rec_buf, "tar": tar_buf, "lab": lab_buf})

    _last_run["nc"] = nc
    _last_run["in_maps"] = in_maps
    _last_run["key"] = key
    res = None
    last_err = None
    for _attempt in range(3):  # the device occasionally faults transiently
        try:
            res = run_bass_kernel_spmd(nc, in_maps, list(range(NCORES)))
            break
        except Exception as e:  # noqa: BLE001
            last_err = e
            import time as _time
            _time.sleep(2.0)
    if res is None:
        raise last_err

    # host: fold diagonal blocks -> [B, C] sums/counts, then final reduction
    counts = np.zeros((B, C), dtype=np.float64)
    sums = np.zeros((B, C), dtype=np.float64)
    jj = np.arange(GROUP)
    for m in range(NCORES):
        o = res.results[m]["out"].astype(np.float64)  # [128, R*128]
        for r in range(R):
            b = m * R + r
            P = o[:, r * 128:(r + 1) * 128]
            # real data sits in the 8 diagonal [16,16] blocks (j==j')
            blocks = P.reshape(GROUP, HI, GROUP, 2 * LOB)[jj, :, jj, :]
            folded = blocks.sum(axis=0)  # [16 (H), 16 (L|8+L)]
            counts[b] = folded[:, :LOB].reshape(C)
            sums[b] = folded[:, LOB:].reshape(C)

    present = counts > 0
    means = np.where(present, sums / np.where(present, counts, 1.0), 0.0)
    pmask = present.astype(np.float64)
    n_clusters_b = pmask.sum(axis=1)
    b_present = n_clusters_b > 0
    batch_loss = (means * pmask).sum(axis=1) / np.where(b_present, n_clusters_b, 1.0)
    n_b = b_present.sum()
    loss = np.where(b_present, batch_loss, 0.0).sum() / max(n_b, 1)
    return np.float32(loss)



# revision 2
# speedup vs baseline: 7.9700x; 7.9700x over previous
"""Trainium2 Bass kernel for nn_AutoEncoderLoss (two-level segment-mean MSE).

Strategy
--------
The loss only needs per-(batch, cluster) sums of (reco-target)^2 and counts.
Counts and the grouping are a pure function of the index tensors, so the host
computes the layout: a stable argsort by fused segment id s = b*C + c places
every segment's points contiguously; each segment is padded up to a whole
number of 128-point columns and laid out as a [128, w_s] column block.
Segments are sharded 512-per-core (4 whole batches per core, matching the
data-parallel hint) and concatenated along the free dim into a [128, T] tile
per core (~6% padding overhead), stored tile-major so every DMA is a fully
contiguous 256 KB block.

The device does all the O(N) math: stream rec/tar tiles, d = rec - tar (DVE),
v = d^2 (ScalarE, bf16 out), then per-column partition sums via a ones-vector
matmul on the PE (out[1, tw] = ones[128,1]^T @ v[128, tw], f32 PSUM). Column
sums are copied to SBUF and DMA'd out once per iteration: [1, T] f32 per core.

The host then folds each segment's w_s column sums (cumsum-diff, float64),
takes counts from the same bincount that defined the layout, and runs the
reference's O(B*C) masked two-level mean. Zero padding is exact: pad slots
have rec = tar = 0 so they contribute 0 to every sum.

This makes the kernel memory-roofline bound: per core 2 tensors x 128 x T x
4 B ~= 8.9 MB at 358 GB/s ~= 25 us, with DVE/ScalarE/PE each under 8 us and
overlapped via double-buffered tile pools.
"""

import os as _os
import numpy as np
from contextlib import ExitStack

NCORES = 8
B_HC = 32            # hardcoded problem shape (asserted at runtime)
C_HC = 128
SEG_PER_CORE = B_HC * C_HC // NCORES  # 512
TW = 512             # columns per tile = one PSUM bank of f32 column sums
IN_DTYPE = _os.environ.get("K_IN_DTYPE", "f32")  # "f32" | "bf16"

_prog_cache = {}
_last_run = {}


def _build_program(n_tiles, repeat=None, internal_inputs=False, stage="full"):
    """Build + compile the SPMD bass program for n_tiles tiles of [128, TW].

    repeat: wrap the compute in a hardware For_i loop (profiling).
    internal_inputs: inputs become Internal DRAM scratch (no host transfer);
    timing is data-independent, used only for profiling.
    stage: "dma" | "sub" | "sq" | "mm" | "full" - truncate the per-tile
    pipeline after that stage (engine attribution without perfetto).
    """
    import concourse.tile as tile
    from concourse import bacc, mybir

    f32 = mybir.dt.float32
    bf16 = mybir.dt.bfloat16
    AT = mybir.ActivationFunctionType
    in_dt = bf16 if IN_DTYPE == "bf16" else f32

    nc = bacc.Bacc("TRN2", target_bir_lowering=False, debug=False,
                   num_devices=NCORES)
    in_kind = "Internal" if internal_inputs else "ExternalInput"
    rec = nc.dram_tensor("rec", [n_tiles * 128, TW], in_dt, kind=in_kind).ap()
    tar = nc.dram_tensor("tar", [n_tiles * 128, TW], in_dt, kind=in_kind).ap()
    out = nc.dram_tensor("out", [1, n_tiles * TW], f32,
                         kind="ExternalOutput").ap()

    with tile.TileContext(nc) as tc, ExitStack() as ctx:
        io_pool = ctx.enter_context(tc.tile_pool(name="io", bufs=3))
        tmp_pool = ctx.enter_context(tc.tile_pool(name="tmp", bufs=3))
        one_pool = ctx.enter_context(tc.tile_pool(name="one", bufs=1))
        psum_pool = ctx.enter_context(tc.tile_pool(name="ps", bufs=4,
                                                   space="PSUM"))
        out_pool = ctx.enter_context(tc.tile_pool(name="ob", bufs=1))

        ones = one_pool.tile([128, 1], bf16, tag="ones")
        nc.vector.memset(ones[:], 1.0)
        ob = out_pool.tile([1, n_tiles * TW], f32, tag="ob")

        if repeat is not None:
            ctx.enter_context(tc.For_i(0, repeat, 1))

        for t in range(n_tiles):
            rec_t = io_pool.tile([128, TW], in_dt, tag="rec")
            nc.sync.dma_start(out=rec_t[:], in_=rec[t * 128:(t + 1) * 128, :])
            tar_t = io_pool.tile([128, TW], in_dt, tag="tar")
            nc.sync.dma_start(out=tar_t[:], in_=tar[t * 128:(t + 1) * 128, :])
            if stage == "dma":
                continue
            d_t = tmp_pool.tile([128, TW], in_dt, tag="d")
            nc.vector.tensor_sub(d_t[:], rec_t[:], tar_t[:])
            if stage == "sub":
                continue
            v_t = tmp_pool.tile([128, TW], bf16, tag="v")
            nc.scalar.activation(v_t[:], d_t[:], AT.Square)
            if stage == "sq":
                continue
            ps = psum_pool.tile([1, TW], f32, tag="ps")
            nc.tensor.matmul(ps[:], ones[:], v_t[:], start=True, stop=True)
            if stage == "mm":
                continue
            nc.vector.tensor_copy(ob[:, t * TW:(t + 1) * TW], ps[:])

        if stage == "full":
            nc.sync.dma_start(out=out[:], in_=ob[:])

    nc.compile()
    return nc


def _cast_in(arr_f32):
    if IN_DTYPE == "bf16":
        import ml_dtypes
        return arr_f32.astype(ml_dtypes.bfloat16)
    return arr_f32


def kernel(reco, target, clabel, batch_index, num_batches, num_clusters):
    from concourse.bass_utils import run_bass_kernel_spmd

    B = int(num_batches)
    C = int(num_clusters)
    assert B == B_HC and C == C_HC, f"kernel hardcoded for B=32,C=128, got {B},{C}"
    nseg = B * C

    rec = np.ascontiguousarray(np.asarray(reco, dtype=np.float32).reshape(-1))
    tar = np.ascontiguousarray(np.asarray(target, dtype=np.float32).reshape(-1))
    cl = np.asarray(clabel).astype(np.int32).reshape(-1)
    bi = np.asarray(batch_index).astype(np.int32).reshape(-1)
    N = rec.shape[0]

    # host layout: group points by fused segment id (stable counting order)
    key = bi * np.int32(C) + cl                      # [N] in [0, 4096)
    order = np.argsort(key, kind="stable")
    key_s = key[order]
    counts = np.bincount(key, minlength=nseg).astype(np.int64)  # [B*C]
    w = (counts + 127) >> 7                          # columns per segment
    w_pc = w.reshape(NCORES, SEG_PER_CORE)
    colbase_pc = np.zeros((NCORES, SEG_PER_CORE), dtype=np.int64)
    colbase_pc[:, 1:] = np.cumsum(w_pc[:, :-1], axis=1)
    T_core = w_pc.sum(axis=1)                        # used cols per core
    n_tiles = max(1, int(-(-int(T_core.max()) // TW)))
    T = n_tiles * TW

    # destination slot of each (sorted) point: core, partition, column
    seg_start = np.zeros(nseg + 1, dtype=np.int64)
    seg_start[1:] = np.cumsum(counts)
    i_loc = np.arange(N, dtype=np.int64) - seg_start[key_s]
    p = i_loc & 127
    col = colbase_pc.reshape(-1)[key_s] + (i_loc >> 7)
    core = key_s >> np.int64(SEG_PER_CORE.bit_length() - 1)  # key_s // 512
    # tile-major per-core layout: [n_tiles, 128, TW] flattened
    dest = (core * (n_tiles * 128) + (col // TW) * 128 + p) * TW + (col % TW)

    if IN_DTYPE == "bf16":
        import ml_dtypes
        rec_buf = np.zeros(NCORES * n_tiles * 128 * TW, dtype=np.uint16)
        tar_buf = np.zeros_like(rec_buf)
        rec_buf[dest] = rec[order].astype(ml_dtypes.bfloat16).view(np.uint16)
        tar_buf[dest] = tar[order].astype(ml_dtypes.bfloat16).view(np.uint16)
        rec_buf = rec_buf.view(ml_dtypes.bfloat16)
        tar_buf = tar_buf.view(ml_dtypes.bfloat16)
    else:
        rec_buf = np.zeros(NCORES * n_tiles * 128 * TW, dtype=np.float32)
        tar_buf = np.zeros_like(rec_buf)
        rec_buf[dest] = rec[order]
        tar_buf[dest] = tar[order]
    rec_buf = rec_buf.reshape(NCORES, n_tiles * 128, TW)
    tar_buf = tar_buf.reshape(NCORES, n_tiles * 128, TW)

    key_cache = n_tiles
    if key_cache not in _prog_cache:
        _prog_cache[key_cache] = _build_program(n_tiles)
    nc = _prog_cache[key_cache]

    in_maps = [{"rec": rec_buf[m], "tar": tar_buf[m]} for m in range(NCORES)]
    _last_run["nc"] = nc
    _last_run["in_maps"] = in_maps
    _last_run["key"] = key_cache

    res = None
    last_err = None
    for _attempt in range(3):  # the device occasionally faults transiently
        try:
            res = run_bass_kernel_spmd(nc, in_maps, list(range(NCORES)))
            break
        except Exception as e:  # noqa: BLE001
            last_err = e
            import time as _time
            _time.sleep(2.0)
    if res is None:
        raise last_err

    # host: fold each segment's column sums, then the O(B*C) final reduction
    sums = np.zeros(nseg, dtype=np.float64)
    for m in range(NCORES):
        colsums = res.results[m]["out"].astype(np.float64).reshape(-1)  # [T]
        cs = np.zeros(T + 1, dtype=np.float64)
        cs[1:] = np.cumsum(colsums)
        s0, s1 = colbase_pc[m], colbase_pc[m] + w_pc[m]
        sums[m * SEG_PER_CORE:(m + 1) * SEG_PER_CORE] = cs[s1] - cs[s0]

    counts_f = counts.astype(np.float64).reshape(B, C)
    sums2 = sums.reshape(B, C)
    present = counts_f > 0
    means = np.where(present, sums2 / np.where(present, counts_f, 1.0), 0.0)
    pmask = present.astype(np.float64)
    n_clusters_b = pmask.sum(axis=1)
    b_present = n_clusters_b > 0
    batch_loss = (means * pmask).sum(axis=1) / np.where(b_present, n_clusters_b, 1.0)
    n_b = b_present.sum()
    loss = np.where(b_present, batch_loss, 0.0).sum() / max(n_b, 1)
    return np.float32(loss)


def profile_hw(np_inputs=None, k1=4, k2=1004, pairs=10, verbose=False):
    """Measure steady-state HW ns per kernel iteration.

    Runs two hardware-loop variants (k1/k2 repeats of the full compute,
    Internal-DRAM inputs so no transfers) in interleaved pairs; the median
    of per-pair wall-clock differences divided by (k2-k1) cancels dispatch
    overhead and is robust to the time-shared device's slow patches.
    """
    import time
    from concourse.bass_utils import run_bass_kernel_spmd
    if not _last_run and np_inputs is not None:
        kernel(**np_inputs)
    n_tiles = _last_run["key"]

    ncs = {}
    for k in (k1, k2):
        ck = ("prof", n_tiles, k, "full")
        if ck not in _prog_cache:
            _prog_cache[ck] = _build_program(n_tiles, repeat=k,
                                             internal_inputs=True)
        ncs[k] = _prog_cache[ck]

    def one(k):
        t0 = time.time()
        run_bass_kernel_spmd(ncs[k], [{} for _ in range(NCORES)],
                             list(range(NCORES)))
        return time.time() - t0

    one(k1)  # warm both NEFFs
    one(k2)
    diffs = []
    for _ in range(pairs):
        try:
            ta = one(k1)
            tb = one(k2)
        except Exception:  # transient device flake: skip pair
            time.sleep(2)
            continue
        diffs.append((tb - ta) / (k2 - k1) * 1e9)
    diffs.sort()
    if verbose:
        print("pair diffs (ns/iter):", [f"{d:.0f}" for d in diffs])
    return diffs[len(diffs) // 2] if diffs else float("nan")


def profile_stages(np_inputs=None, k1=4, k2=204, samples=4):
    """Per-stage steady-state times (us): dma, +sub, +sq, +mm, full."""
    if not _last_run and np_inputs is not None:
        kernel(**np_inputs)
    out = {}
    for stage in ("dma", "sub", "sq", "mm", "full"):
        t1 = _timed_prof(k1, stage, samples)
        t2 = _timed_prof(k2, stage, samples)
        out[stage] = (t2 - t1) / (k2 - k1) * 1e6
    return out


def _timed_prof(k, stage, samples):
    import time
    from concourse.bass_utils import run_bass_kernel_spmd
    n_tiles = _last_run["key"]
    ck = ("prof", n_tiles, k, stage)
    if ck not in _prog_cache:
        _prog_cache[ck] = _build_program(n_tiles, repeat=k,
                                         internal_inputs=True, stage=stage)
    nc = _prog_cache[ck]
    best = float("inf")
    for _ in range(samples):
        t0 = time.time()
        run_bass_kernel_spmd(nc, [{} for _ in range(NCORES)],
                             list(range(NCORES)))
        best = min(best, time.time() - t0)
    return best


# revision 11
# speedup vs baseline: 7.9875x; 1.0022x over previous
"""Trainium2 Bass kernel for nn_AutoEncoderLoss (two-level segment-mean MSE).

Strategy
--------
The loss only needs per-(batch, cluster) sums of (reco-target)^2 and counts.
Counts and the grouping are a pure function of the index tensors, so the host
computes the layout: a stable argsort by fused segment id s = b*C + c places
every segment's points contiguously; each segment is padded up to a whole
number of 128-point columns and laid out as a [128, w_s] column block.
Segments are sharded 512-per-core (4 whole batches per core, matching the
data-parallel hint) and concatenated along the free dim into a [128, T] tile
per core (~6% padding overhead), stored tile-major so every DMA is a fully
contiguous 256 KB block.

The device does all the O(N) math: stream rec/tar tiles, d = rec - tar (DVE),
v = d^2 (ScalarE, bf16 out), then per-column partition sums via a ones-vector
matmul on the PE (out[1, tw] = ones[128,1]^T @ v[128, tw], f32 PSUM). Column
sums are copied to SBUF and DMA'd out once per iteration: [1, T] f32 per core.

The host then folds each segment's w_s column sums (cumsum-diff, float64),
takes counts from the same bincount that defined the layout, and runs the
reference's O(B*C) masked two-level mean. Zero padding is exact: pad slots
have rec = tar = 0 so they contribute 0 to every sum.

This makes the kernel memory-roofline bound: per core 2 tensors x 128 x T x
4 B ~= 8.9 MB at 358 GB/s ~= 25 us, with DVE/ScalarE/PE each under 8 us and
overlapped via double-buffered tile pools.
"""

import os as _os
import numpy as np
from contextlib import ExitStack

NCORES = 8
B_HC = 32            # hardcoded problem shape (asserted at runtime)
C_HC = 128
SEG_PER_CORE = B_HC * C_HC // NCORES  # 512
TW = 512             # columns per tile = one PSUM bank of f32 column sums
IN_DTYPE = _os.environ.get("K_IN_DTYPE", "f32")  # "f32" | "bf16"
DMA_SPLIT = int(_os.environ.get("K_DMA_SPLIT", "1"))  # input DMA queues: 1|2|4

_prog_cache = {}
_last_run = {}


def _build_program(n_tiles, repeat=None, internal_inputs=False, stage="full"):
    """Build + compile the SPMD bass program for n_tiles tiles of [128, TW].

    repeat: wrap the compute in a hardware For_i loop (profiling).
    internal_inputs: inputs become Internal DRAM scratch (no host transfer);
    timing is data-independent, used only for profiling.
    stage: "dma" | "sub" | "sq" | "mm" | "full" - truncate the per-tile
    pipeline after that stage (engine attribution without perfetto).
    """
    import concourse.tile as tile
    from concourse import bacc, mybir

    f32 = mybir.dt.float32
    bf16 = mybir.dt.bfloat16
    AT = mybir.ActivationFunctionType
    in_dt = {"f32": f32, "bf16": bf16, "f8": mybir.dt.float8e4}[IN_DTYPE]

    nc = bacc.Bacc("TRN2", target_bir_lowering=False, debug=False,
                   num_devices=NCORES)
    in_kind = "Internal" if internal_inputs else "ExternalInput"
    rec = nc.dram_tensor("rec", [n_tiles * 128, TW], in_dt, kind=in_kind).ap()
    tar = nc.dram_tensor("tar", [n_tiles * 128, TW], in_dt, kind=in_kind).ap()
    out = nc.dram_tensor("out", [1, n_tiles * TW], f32,
                         kind="ExternalOutput").ap()

    with tile.TileContext(nc) as tc, ExitStack() as ctx:
        io_pool = ctx.enter_context(tc.tile_pool(name="io", bufs=3))
        tmp_pool = ctx.enter_context(tc.tile_pool(name="tmp", bufs=3))
        one_pool = ctx.enter_context(tc.tile_pool(name="one", bufs=1))
        psum_pool = ctx.enter_context(tc.tile_pool(name="ps", bufs=4,
                                                   space="PSUM"))
        out_pool = ctx.enter_context(tc.tile_pool(name="ob", bufs=1))

        ones = one_pool.tile([128, 1], bf16, tag="ones")
        nc.vector.memset(ones[:], 1.0)
        ob = out_pool.tile([1, n_tiles * TW], f32, tag="ob")

        if repeat is not None:
            ctx.enter_context(tc.For_i(0, repeat, 1))

        # input DMAs spread over up to 4 engine HW-DGE queues; each engine's
        # trigger stream is cheap, the point is parallel DMA rings
        q = [nc.sync, nc.scalar, nc.vector, nc.tensor][:max(1, DMA_SPLIT)]

        for t in range(n_tiles):
            rec_t = io_pool.tile([128, TW], in_dt, tag="rec")
            tar_t = io_pool.tile([128, TW], in_dt, tag="tar")
            if DMA_SPLIT <= 2:
                q[0].dma_start(out=rec_t[:], in_=rec[t * 128:(t + 1) * 128, :])
                q[-1].dma_start(out=tar_t[:], in_=tar[t * 128:(t + 1) * 128, :])
            else:
                q[0].dma_start(out=rec_t[0:64, :], in_=rec[t * 128:t * 128 + 64, :])
                q[1].dma_start(out=rec_t[64:128, :], in_=rec[t * 128 + 64:(t + 1) * 128, :])
                q[2].dma_start(out=tar_t[0:64, :], in_=tar[t * 128:t * 128 + 64, :])
                q[3].dma_start(out=tar_t[64:128, :], in_=tar[t * 128 + 64:(t + 1) * 128, :])
            if stage == "dma":
                continue
            d_dt = bf16 if IN_DTYPE == "f8" else in_dt
            d_t = tmp_pool.tile([128, TW], d_dt, tag="d")
            nc.vector.tensor_sub(d_t[:], rec_t[:], tar_t[:])
            if stage == "sub":
                continue
            v_t = tmp_pool.tile([128, TW], bf16, tag="v")
            nc.scalar.activation(v_t[:], d_t[:], AT.Square)
            if stage == "sq":
                continue
            ps = psum_pool.tile([1, TW], f32, tag="ps")
            nc.tensor.matmul(ps[:], ones[:], v_t[:], start=True, stop=True)
            if stage == "mm":
                continue
            nc.vector.tensor_copy(ob[:, t * TW:(t + 1) * TW], ps[:])

        if stage == "full":
            nc.sync.dma_start(out=out[:], in_=ob[:])

    nc.compile()
    return nc


def _cast_in(arr_f32):
    if IN_DTYPE == "bf16":
        import ml_dtypes
        return arr_f32.astype(ml_dtypes.bfloat16)
    return arr_f32


def kernel(reco, target, clabel, batch_index, num_batches, num_clusters):
    from concourse.bass_utils import run_bass_kernel_spmd

    B = int(num_batches)
    C = int(num_clusters)
    assert B == B_HC and C == C_HC, f"kernel hardcoded for B=32,C=128, got {B},{C}"
    nseg = B * C

    rec = np.ascontiguousarray(np.asarray(reco, dtype=np.float32).reshape(-1))
    tar = np.ascontiguousarray(np.asarray(target, dtype=np.float32).reshape(-1))
    cl = np.asarray(clabel).astype(np.int32).reshape(-1)
    bi = np.asarray(batch_index).astype(np.int32).reshape(-1)
    N = rec.shape[0]

    # host layout: group points by fused segment id (stable counting order)
    key = bi * np.int32(C) + cl                      # [N] in [0, 4096)
    order = np.argsort(key, kind="stable")
    key_s = key[order]
    counts = np.bincount(key, minlength=nseg).astype(np.int64)  # [B*C]
    w = (counts + 127) >> 7                          # columns per segment
    w_pc = w.reshape(NCORES, SEG_PER_CORE)
    colbase_pc = np.zeros((NCORES, SEG_PER_CORE), dtype=np.int64)
    colbase_pc[:, 1:] = np.cumsum(w_pc[:, :-1], axis=1)
    T_core = w_pc.sum(axis=1)                        # used cols per core
    n_tiles = max(1, int(-(-int(T_core.max()) // TW)))
    T = n_tiles * TW

    # destination slot of each (sorted) point: core, partition, column
    seg_start = np.zeros(nseg + 1, dtype=np.int64)
    seg_start[1:] = np.cumsum(counts)
    i_loc = np.arange(N, dtype=np.int64) - seg_start[key_s]
    p = i_loc & 127
    col = colbase_pc.reshape(-1)[key_s] + (i_loc >> 7)
    core = key_s >> np.int64(SEG_PER_CORE.bit_length() - 1)  # key_s // 512
    # tile-major per-core layout: [n_tiles, 128, TW] flattened
    dest = (core * (n_tiles * 128) + (col // TW) * 128 + p) * TW + (col % TW)

    if IN_DTYPE == "f32":
        rec_buf = np.zeros(NCORES * n_tiles * 128 * TW, dtype=np.float32)
        tar_buf = np.zeros_like(rec_buf)
        rec_buf[dest] = rec[order]
        tar_buf[dest] = tar[order]
    else:
        import ml_dtypes
        cdt = ml_dtypes.bfloat16 if IN_DTYPE == "bf16" else ml_dtypes.float8_e4m3
        udt = np.uint16 if IN_DTYPE == "bf16" else np.uint8
        rec_buf = np.zeros(NCORES * n_tiles * 128 * TW, dtype=udt)
        tar_buf = np.zeros_like(rec_buf)
        rec_buf[dest] = rec[order].astype(cdt).view(udt)
        tar_buf[dest] = tar[order].astype(cdt).view(udt)
        rec_buf = rec_buf.view(cdt)
        tar_buf = tar_buf.view(cdt)
    rec_buf = rec_buf.reshape(NCORES, n_tiles * 128, TW)
    tar_buf = tar_buf.reshape(NCORES, n_tiles * 128, TW)

    key_cache = (n_tiles, IN_DTYPE, DMA_SPLIT)
    if key_cache not in _prog_cache:
        _prog_cache[key_cache] = _build_program(n_tiles)
    nc = _prog_cache[key_cache]

    in_maps = [{"rec": rec_buf[m], "tar": tar_buf[m]} for m in range(NCORES)]
    _last_run["nc"] = nc
    _last_run["in_maps"] = in_maps
    _last_run["key"] = key_cache
    _last_run["n_tiles"] = n_tiles

    res = None
    last_err = None
    for _attempt in range(3):  # the device occasionally faults transiently
        try:
            res = run_bass_kernel_spmd(nc, in_maps, list(range(NCORES)))
            break
        except Exception as e:  # noqa: BLE001
            last_err = e
            import time as _time
            _time.sleep(2.0)
    if res is None:
        raise last_err

    # host: fold each segment's column sums, then the O(B*C) final reduction
    sums = np.zeros(nseg, dtype=np.float64)
    for m in range(NCORES):
        colsums = res.results[m]["out"].astype(np.float64).reshape(-1)  # [T]
        cs = np.zeros(T + 1, dtype=np.float64)
        cs[1:] = np.cumsum(colsums)
        s0, s1 = colbase_pc[m], colbase_pc[m] + w_pc[m]
        sums[m * SEG_PER_CORE:(m + 1) * SEG_PER_CORE] = cs[s1] - cs[s0]

    counts_f = counts.astype(np.float64).reshape(B, C)
    sums2 = sums.reshape(B, C)
    present = counts_f > 0
    means = np.where(present, sums2 / np.where(present, counts_f, 1.0), 0.0)
    pmask = present.astype(np.float64)
    n_clusters_b = pmask.sum(axis=1)
    b_present = n_clusters_b > 0
    batch_loss = (means * pmask).sum(axis=1) / np.where(b_present, n_clusters_b, 1.0)
    n_b = b_present.sum()
    loss = np.where(b_present, batch_loss, 0.0).sum() / max(n_b, 1)
    return np.float32(loss)


def profile_hw(np_inputs=None, k1=4, k2=1004, pairs=10, verbose=False):
    """Measure steady-state HW ns per kernel iteration.

    Runs two hardware-loop variants (k1/k2 repeats of the full compute,
    Internal-DRAM inputs so no transfers) in interleaved pairs; the median
    of per-pair wall-clock differences divided by (k2-k1) cancels dispatch
    overhead and is robust to the time-shared device's slow patches.
    """
    import time
    from concourse.bass_utils import run_bass_kernel_spmd
    if not _last_run and np_inputs is not None:
        kernel(**np_inputs)
    n_tiles = _last_run["n_tiles"]

    ncs = {}
    for k in (k1, k2):
        ck = ("prof", n_tiles, k, "full", IN_DTYPE, DMA_SPLIT)
        if ck not in _prog_cache:
            _prog_cache[ck] = _build_program(n_tiles, repeat=k,
                                             internal_inputs=True)
        ncs[k] = _prog_cache[ck]

    def one(k):
        t0 = time.time()
        run_bass_kernel_spmd(ncs[k], [{} for _ in range(NCORES)],
                             list(range(NCORES)))
        return time.time() - t0

    one(k1)  # warm both NEFFs
    one(k2)
    diffs = []
    for _ in range(pairs):
        try:
            ta = one(k1)
            tb = one(k2)
        except Exception:  # transient device flake: skip pair
            time.sleep(2)
            continue
        diffs.append((tb - ta) / (k2 - k1) * 1e9)
    diffs.sort()
    if verbose:
        print("pair diffs (ns/iter):", [f"{d:.0f}" for d in diffs])
    return diffs[len(diffs) // 2] if diffs else float("nan")


def profile_stages(np_inputs=None, k1=4, k2=1004, pairs=6):
    """Per-stage steady-state times (us): dma, +sub, +sq, +mm, full.

    Same interleaved-pair median methodology as profile_hw, per stage.
    """
    import time
    from concourse.bass_utils import run_bass_kernel_spmd
    if not _last_run and np_inputs is not None:
        kernel(**np_inputs)
    n_tiles = _last_run["n_tiles"]
    out = {}
    for stage in ("dma", "sub", "sq", "mm", "full"):
        ncs = {}
        for k in (k1, k2):
            ck = ("prof", n_tiles, k, stage, IN_DTYPE, DMA_SPLIT)
            if ck not in _prog_cache:
                _prog_cache[ck] = _build_program(n_tiles, repeat=k,
                                                 internal_inputs=True,
                                                 stage=stage)
            ncs[k] = _prog_cache[ck]

        def one(k):
            t0 = time.time()
            run_bass_kernel_spmd(ncs[k], [{} for _ in range(NCORES)],
                                 list(range(NCORES)))
            return time.time() - t0

        one(k1)
        one(k2)
        diffs = []
        for _ in range(pairs):
            try:
                ta = one(k1)
                tb = one(k2)
            except Exception:
                time.sleep(2)
                continue
            diffs.append((tb - ta) / (k2 - k1) * 1e6)
        diffs.sort()
        out[stage] = diffs[len(diffs) // 2] if diffs else float("nan")
    return out
